# revision 11
# baseline (speedup 1.0000x reference)
"""Causal multi-head self-attention on 8 TRN2 NeuronCores.

Sharding: tensor-parallel over heads. 16 heads / 8 cores = 2 heads per core.
Each core computes q/k/v projections for its 2 heads, block-causal
attention, and a partial output projection against its 128-column slice of
W_O. The host sums the 8 partial outputs.

v2 (perf rework vs the fp32r baseline):
  * all matmul operands bf16 (halves LDWEIGHTS via FWL, halves x/out DMA;
    PSUM accumulation stays fp32 so rel-err stays ~1e-3, tolerance is 2e-2)
  * scores run as two row-tiled K=64 matmuls (tile_position (0,0)/(64,0))
    that share one PE pass, instead of K=128 zero-padded pairs
  * softmax reciprocal on [2,512] via reciprocal_approx_fast BEFORE the
    indicator-matmul broadcast (kills the 3.4us-per-qtile DVE reciprocal)
  * diag tri-masks moved to the (idle) GpSimd engine
  * v-transpose lands in a [vA|ones|vB|ones] 130-wide slot with one strided
    copy; softmax sums still come free from the ones columns in the av lhsT

Layouts on core c (heads 2c, 2c+1 = "A", "B"):
  qT/kT  [128, 2048]  feature-major bf16; rows 0:64 head A dk, 64:128 B
  vtok   [128, 16, 130] token-major v + ones at cols 64 (A) / 129 (B)
  sAB    [128, 512+w] PSUM: A scores at 0:w, B at 512:512+w (bank-aligned)
  eAB    bf16 exp'd scores (ScalarE), junk gap [w:512] unread on diag tiles
  ctxAB  [65, 2*512] PSUM per head pair; row 64 = softmax sums
  out    partial [1024, 8192] feature-major bf16; host sums over cores

The attention k-tile loop is software-pipelined (ctx lags scores by LAG
tiles); normalization + output projection of q-tile n are deferred into
tile n+1's matmul stream so the PE never waits on ScalarE/DVE.
"""

import numpy as np
import ml_dtypes
from contextlib import ExitStack

import concourse.bass as bass
import concourse.tile as tile
from concourse import bacc, mybir
from concourse.bass_utils import run_bass_kernel_spmd

F32 = mybir.dt.float32
F32R = mybir.dt.float32r
BF16 = mybir.dt.bfloat16
BF = ml_dtypes.bfloat16

B, S, D, H = 4, 2048, 1024, 16
DK = D // H  # 64
NCORES = 8
T = B * S  # 8192 tokens
KT = D // 128  # 8 contraction tiles for projections
QTILE = 512  # q-tile width (tokens)
KTILE = 128  # k-tile width (tokens)
NQT = S // QTILE  # 4 q-tiles per batch
NKT = S // KTILE  # 16 k-tiles per batch
LAG = 3  # ctx matmuls trail scores by this many k-tiles
EXP_FUNC = mybir.ActivationFunctionType.Exp
INV_SQRT_DK = 1.0 / np.sqrt(DK)


def build_nc():
    nc = bacc.Bacc("TRN2", target_bir_lowering=False, debug=False)

    xT = nc.dram_tensor("xT", [D, T], BF16, kind="ExternalInput").ap()
    wq = nc.dram_tensor("wq", [D, 128], BF16, kind="ExternalInput").ap()
    wk = nc.dram_tensor("wk", [D, 128], BF16, kind="ExternalInput").ap()
    wv = nc.dram_tensor("wv", [D, 128], BF16, kind="ExternalInput").ap()
    wo = nc.dram_tensor("wo", [128, D], BF16, kind="ExternalInput").ap()
    tri = nc.dram_tensor("tri", [128, 128], BF16, kind="ExternalInput").ap()
    ind = nc.dram_tensor("ind", [2, 128], BF16, kind="ExternalInput").ap()
    ident = nc.dram_tensor("ident", [128, 128], BF16, kind="ExternalInput").ap()
    outT = nc.dram_tensor("outT", [D, T], BF16, kind="ExternalOutput").ap()

    with ExitStack() as ctx:
        tc = ctx.enter_context(tile.TileContext(nc))
        consts = ctx.enter_context(tc.tile_pool(name="consts", bufs=1))
        xt_pool = ctx.enter_context(tc.tile_pool(name="xt_pool", bufs=2))
        batch_pool = ctx.enter_context(tc.tile_pool(name="batch_pool", bufs=2))
        vtmp_pool = ctx.enter_context(tc.tile_pool(name="vtmp_pool", bufs=3))
        exp_pool = ctx.enter_context(tc.tile_pool(name="exp_pool", bufs=4))
        ctxn_pool = ctx.enter_context(tc.tile_pool(name="ctxn_pool", bufs=2))
        tmpb_pool = ctx.enter_context(tc.tile_pool(name="tmpb_pool", bufs=2))
        oall_pool = ctx.enter_context(tc.tile_pool(name="oall_pool", bufs=2))
        small_pool = ctx.enter_context(tc.tile_pool(name="small_pool", bufs=2))
        ps = ctx.enter_context(tc.tile_pool(name="ps", bufs=1, space="PSUM"))

        # --- constants / weights (persistent) ---
        wq_sb = consts.tile([128, KT, 128], BF16)
        nc.sync.dma_start(out=wq_sb, in_=wq.rearrange("(kt p) m -> p kt m", p=128))
        wk_sb = consts.tile([128, KT, 128], BF16)
        nc.sync.dma_start(out=wk_sb, in_=wk.rearrange("(kt p) m -> p kt m", p=128))
        wv_sb = consts.tile([128, KT, 128], BF16)
        nc.sync.dma_start(out=wv_sb, in_=wv.rearrange("(kt p) m -> p kt m", p=128))
        wo_sb = consts.tile([128, KT, 128], BF16)
        nc.sync.dma_start(out=wo_sb, in_=wo.rearrange("p (jt m) -> p jt m", jt=KT))
        tri_sb = consts.tile([128, 128], BF16)
        nc.sync.dma_start(out=tri_sb, in_=tri)
        ind_sb = consts.tile([2, 128], BF16)
        nc.sync.dma_start(out=ind_sb, in_=ind)
        ident_sb = consts.tile([128, 128], BF16)
        nc.sync.dma_start(out=ident_sb, in_=ident)

        pending = []
        for b in range(B):
            tb = b * S  # global token base of this batch

            # --- stage A: q/k/v projections (feature-major) + v transpose ---
            qT_sb = batch_pool.tile([128, S], BF16, name="qT_sb")
            kT_sb = batch_pool.tile([128, S], BF16, name="kT_sb")
            # vtok slot: [vA(64) | onesA(1) | vB(64) | onesB(1)]; av lhsT for
            # head A is cols 0:65, head B cols 65:130 (sums ride along free).
            vtok_sb = batch_pool.tile([128, NKT, 130], BF16, name="vtok_sb")
            nc.vector.memset(vtok_sb[:, :, 64:65], 1.0)
            nc.vector.memset(vtok_sb[:, :, 129:130], 1.0)

            for tt in range(NQT):
                t0 = tb + tt * QTILE
                xt = xt_pool.tile([128, KT, QTILE], BF16, name="xt", tag="xt")
                nc.sync.dma_start(
                    out=xt,
                    in_=xT.rearrange("(kt p) t -> p kt t", p=128)[
                        :, :, t0 : t0 + QTILE
                    ],
                )

                qP = ps.tile([128, QTILE], F32, name="qP", tag="mm", bufs=2)
                for kt in range(KT):
                    nc.tensor.matmul(
                        qP, wq_sb[:, kt, :], xt[:, kt, :], start=(kt == 0), stop=(kt == KT - 1)
                    )
                nc.vector.tensor_copy(qT_sb[:, tt * QTILE : (tt + 1) * QTILE], qP)

                kP = ps.tile([128, QTILE], F32, name="kP", tag="mm", bufs=2)
                for kt in range(KT):
                    nc.tensor.matmul(
                        kP, wk_sb[:, kt, :], xt[:, kt, :], start=(kt == 0), stop=(kt == KT - 1)
                    )
                nc.scalar.copy(kT_sb[:, tt * QTILE : (tt + 1) * QTILE], kP)

                vP = ps.tile([128, QTILE], F32, name="vP", tag="mm", bufs=2)
                for kt in range(KT):
                    nc.tensor.matmul(
                        vP, wv_sb[:, kt, :], xt[:, kt, :], start=(kt == 0), stop=(kt == KT - 1)
                    )
                vT_tmp = vtmp_pool.tile([128, QTILE], BF16, name="vT_tmp")
                nc.vector.tensor_copy(vT_tmp, vP)
                # drain some deferred work here too, but keep a backlog so the
                # first q-tile's exp-latency window still has PE filler
                for _ in range(4):
                    if len(pending) > 4:
                        pending.pop(0)()
                for s in range(QTILE // 128):
                    vtokP = ps.tile([128, 128], BF16, name="vtokP", tag="mm", bufs=2)
                    nc.tensor.transpose(
                        vtokP, vT_tmp[:, s * 128 : (s + 1) * 128], ident_sb
                    )
                    m = tt * 4 + s
                    # one strided copy: head A dk rows -> cols 0:64, head B
                    # rows -> cols 65:129 (skipping the ones column at 64)
                    nc.vector.tensor_copy(
                        vtok_sb[:, m, 0:130].rearrange("p (g c) -> p g c", g=2)[
                            :, :, 0:64
                        ],
                        vtokP.rearrange("p (g c) -> p g c", g=2),
                    )

            # --- stage B: attention per q-tile (SW-pipelined over k-tiles;
            # normalization + output projection of tile n deferred into
            # tile n+1's matmul stream) ---
            for qi in range(NQT):
                q0 = qi * QTILE  # batch-local q base
                nk = 4 * qi + 4  # k-tiles for this q-tile (block-causal)
                ctxAB = ps.tile(
                    [128, 2 * QTILE], F32, name="ctxAB", tag="ctx", bufs=1
                )

                def geom(m, qi=qi):
                    d_off = m - 4 * qi
                    if d_off >= 0:
                        return QTILE - 128 * d_off, 128 * d_off, True
                    return QTILE, 0, False

                exps = {}
                for i in range(nk + LAG):
                    # feed the PE (and DVE) deferred work from the previous
                    # q-tile so the in-order engine queues never sit empty
                    # while an av matmul waits on ScalarE's exp
                    for _ in range(2):
                        if pending:
                            pending.pop(0)()
                    if i < nk:
                        m = i
                        width, qoff, diag = geom(m)
                        sAB = ps.tile(
                            [128, 2 * QTILE], F32, name="sAB", tag="sc", bufs=2
                        )
                        # two row-tiled K=64 matmuls share one PE pass:
                        # head A in array rows 0:63, head B in rows 64:127
                        nc.tensor.matmul(
                            sAB[:, 0:width],
                            kT_sb[0:64, m * 128 : (m + 1) * 128],
                            qT_sb[0:64, q0 + qoff : q0 + QTILE],
                            start=True,
                            stop=True,
                        )
                        nc.tensor.matmul(
                            sAB[:, QTILE : QTILE + width],
                            kT_sb[64:128, m * 128 : (m + 1) * 128],
                            qT_sb[64:128, q0 + qoff : q0 + QTILE],
                            start=True,
                            stop=True,
                        )
                        # one wide exp covers both heads; the gap region
                        # [width:QTILE] holds unread junk for diag tiles.
                        eAB = exp_pool.tile([128, 2 * QTILE], BF16, name="eAB", tag="exp")
                        nc.scalar.activation(
                            eAB[:, 0 : QTILE + width],
                            sAB[:, 0 : QTILE + width],
                            EXP_FUNC,
                            scale=INV_SQRT_DK,
                        )
                        if diag:
                            nc.gpsimd.tensor_mul(eAB[:, 0:128], eAB[:, 0:128], tri_sb)
                            nc.gpsimd.tensor_mul(
                                eAB[:, QTILE : QTILE + 128],
                                eAB[:, QTILE : QTILE + 128],
                                tri_sb,
                            )
                        exps[m] = eAB

                    j = i - LAG
                    if j >= 0:
                        width, qoff, _ = geom(j)
                        first = j == 0
                        last = j == nk - 1
                        eAB = exps.pop(j)
                        nc.tensor.matmul(
                            ctxAB[0:65, qoff:QTILE],
                            vtok_sb[:, j, 0:65],
                            eAB[:, 0:width],
                            start=first,
                            stop=last,
                            skip_group_check=True,
                        )
                        nc.tensor.matmul(
                            ctxAB[0:65, QTILE + qoff : 2 * QTILE],
                            vtok_sb[:, j, 65:130],
                            eAB[:, QTILE : QTILE + width],
                            start=first,
                            stop=last,
                            skip_group_check=True,
                        )

                # normalization part 1 (immediate, frees the ctx PSUM slots):
                # pull the two sums rows to SBUF, shift them to partitions
                # 0-1 (tiny SBUF->SBUF DMAs), reciprocal on [2,512], copy ctx
                # out of PSUM with head B shifted to partitions 64:128.
                s2 = small_pool.tile([65, 2, QTILE], F32, name="s2")
                nc.vector.tensor_copy(s2[64:65, 0, :], ctxAB[64:65, 0:QTILE])
                nc.vector.tensor_copy(s2[64:65, 1, :], ctxAB[64:65, QTILE : 2 * QTILE])
                nc.sync.dma_start(out=s2[0:1, 0, :], in_=s2[64:65, 0, :])
                nc.sync.dma_start(out=s2[1:2, 0, :], in_=s2[64:65, 1, :])
                r2 = small_pool.tile([2, QTILE], F32, name="r2")
                nc.vector.reciprocal_approx_fast(r2, s2[0:2, 0, :])
                r2b = small_pool.tile([2, QTILE], BF16, name="r2b")
                nc.gpsimd.tensor_copy(r2b, r2)
                ctxn = ctxn_pool.tile([128, QTILE], BF16, name="ctxn")
                nc.vector.tensor_copy(ctxn[0:64, :], ctxAB[0:64, 0:QTILE])
                tmpB = tmpb_pool.tile([64, QTILE], BF16, name="tmpB")
                nc.scalar.copy(tmpB, ctxAB[0:64, QTILE : 2 * QTILE])
                nc.sync.dma_start(out=ctxn[64:128, :], in_=tmpB)

                # deferred work, queued as fine-grained ops: the indicator
                # matmul broadcasts both reciprocals to [128, QTILE] PSUM and
                # ctx is normalized; then 8 output-projection matmuls (ctxn is
                # normalized by the time they pop) and the outT write-back.
                o_all = oall_pool.tile([128, KT, QTILE], BF16, name="o_all")

                def norm_ctx(ctxn=ctxn, r2b=r2b):
                    rbP = ps.tile([128, QTILE], F32, name="rbP", tag="mm", bufs=2)
                    nc.tensor.matmul(rbP, ind_sb, r2b, start=True, stop=True)
                    nc.vector.tensor_mul(ctxn, ctxn, rbP)

                def oproj_jt(jt, ctxn=ctxn, o_all=o_all):
                    oP = ps.tile([128, QTILE], F32, name="oP", tag="mm", bufs=2)
                    nc.tensor.matmul(oP, wo_sb[:, jt, :], ctxn, start=True, stop=True)
                    nc.vector.tensor_copy(o_all[:, jt, :], oP)

                def out_dma(q0=q0, tb=tb, o_all=o_all):
                    nc.sync.dma_start(
                        out=outT.rearrange("(jt p) t -> p jt t", p=128)[
                            :, :, tb + q0 : tb + q0 + QTILE
                        ],
                        in_=o_all,
                    )

                pending.append(norm_ctx)
                for jt in range(KT):
                    pending.append(lambda jt=jt: oproj_jt(jt))
                pending.append(out_dma)

        while pending:
            pending.pop(0)()

    nc.compile()
    return nc


_NC = None


def _get_nc():
    global _NC
    if _NC is None:
        _NC = build_nc()
    return _NC


def make_in_maps(x, W_Q, W_K, W_V, W_O):
    xTh = np.ascontiguousarray(
        np.asarray(x, dtype=np.float32).reshape(T, D).T
    ).astype(BF)
    W_Q = np.asarray(W_Q, dtype=np.float32)
    W_K = np.asarray(W_K, dtype=np.float32)
    W_V = np.asarray(W_V, dtype=np.float32)
    W_O = np.asarray(W_O, dtype=np.float32)
    tri = np.triu(np.ones((128, 128), dtype=np.float32)).astype(BF)  # tri[k,q]=1 iff q>=k
    ind2 = np.zeros((2, 128), dtype=np.float32)
    ind2[0, 0:64] = 1.0
    ind2[1, 64:128] = 1.0
    ind2 = ind2.astype(BF)
    ident = np.eye(128, dtype=np.float32).astype(BF)
    in_maps = []
    for c in range(NCORES):
        sl = slice(c * 128, (c + 1) * 128)
        in_maps.append(
            {
                "xT": xTh,
                "wq": np.ascontiguousarray(W_Q[sl, :].T).astype(BF),
                "wk": np.ascontiguousarray(W_K[sl, :].T).astype(BF),
                "wv": np.ascontiguousarray(W_V[sl, :].T).astype(BF),
                "wo": np.ascontiguousarray(W_O.T[sl, :]).astype(BF),
                "tri": tri,
                "ind": ind2,
                "ident": ident,
            }
        )
    return in_maps


def kernel(x, W_Q, W_K, W_V, W_O, _results_hook=None):
    nc = _get_nc()
    in_maps = make_in_maps(x, W_Q, W_K, W_V, W_O)
    res = run_bass_kernel_spmd(nc, in_maps, list(range(NCORES)))
    if _results_hook is not None:
        _results_hook(res)
    acc = np.zeros((D, T), dtype=np.float64)
    for c in range(NCORES):
        acc += np.asarray(res.results[c]["outT"], dtype=np.float64)
    out = np.ascontiguousarray(acc.T).reshape(B, S, D).astype(np.float32)
    return out
